# revision 48
# baseline (speedup 1.0000x reference)
"""MLA-v2 (multi-head latent attention) forward pass on 8 Trainium2 NeuronCores.

Sharding: core c -> (batch b = c // 4, head-group g = c % 4, 4 heads each).
Data parallel over batch; tensor parallel over heads (W_Q / W_up_K / W_up_V
column-sharded, W_O row-sharded).  The compressed latent c_kv is computed
replicated per core.  Each core emits ONE (S, D) bf16 partial (both head
pairs accumulated on-chip in PSUM); the host sums the 4 partials per batch.

On-chip layout is fully "transposed" (feature dim on partitions, sequence on
the free axis) so that no activation transposes are ever needed:
  Q^T = W_Q^T @ X^T      (PSUM -> SBUF, RoPE applied in-place)
  c^T = W_dkv^T @ X^T
  K^T = W_upK^T @ c^T    (RoPE in-place)
  V   = (c^T slice)^T @ W_upV          -> natural (s, d) tiles (JIT in attn)
  S^T[k, q] = (K^T tile)^T @ Q^T       -> softmax over k via matmul tricks
  O'^T = V_aug^T @ exp(S^T)            -> row 64 of V_aug is ones => denom
  out  = attn0^T @ W_O[0:128] + attn1^T @ W_O[128:256]  (single partial)

All activations and weights are bf16 on chip (PSUM accumulation is f32);
softmax denominators/reciprocals stay f32.  RoPE trick: columns of W_Q /
W_up_K are permuted per head so even/odd pairs become [32 evens | 32 odds]
blocks; dot products are invariant because Q and K share the permutation.
Rotation is three elementwise ops plus a 32-partition-block swap (SBUF->SBUF
DMA).  Softmax skips the max-subtraction: scores are bounded (|s| < ~4).
"""

import numpy as np
import ml_dtypes

import concourse.bass as bass
import concourse.bacc as bacc
import concourse.mybir as mybir
import concourse.tile as tile
from concourse.bass_utils import run_bass_kernel_spmd

F32 = mybir.dt.float32
F32R = mybir.dt.float32r
BF16 = mybir.dt.bfloat16
FP8 = mybir.dt.float8e4

B = 2
S = 2048
D = 1024
H = 16
DH = 64
DC = 256
HPC = 4          # heads per core
GD = HPC * DH    # per-core sharded model dim (256)
N_CORES = 8
NKT = D // 128   # k-tiles over D (8)
NCT = DC // 128  # k-tiles over DC (2)
NST = S // 128   # seq tiles (16)
NQC = S // 512   # 512-wide q chunks (4)


def _build_nc():
    nc = bacc.Bacc("TRN2", target_bir_lowering=False, debug=False,
                   num_devices=N_CORES)

    xt_d = nc.dram_tensor("xt", [D, S], BF16, kind="ExternalInput").ap()
    wq_d = nc.dram_tensor("wq", [D, GD], BF16, kind="ExternalInput").ap()
    wdkv_d = nc.dram_tensor("wdkv", [D, DC], BF16, kind="ExternalInput").ap()
    wupk_d = nc.dram_tensor("wupk", [DC, GD], BF16, kind="ExternalInput").ap()
    wupv_d = nc.dram_tensor("wupv", [DC, GD], BF16, kind="ExternalInput").ap()
    wo_d = nc.dram_tensor("wo", [GD, D], BF16, kind="ExternalInput").ap()
    ctab_d = nc.dram_tensor("ctab", [128, S], BF16, kind="ExternalInput").ap()
    stab_d = nc.dram_tensor("stab", [128, S], BF16, kind="ExternalInput").ap()
    mask_d = nc.dram_tensor("mask", [128, 128], BF16, kind="ExternalInput").ap()
    ee_d = nc.dram_tensor("ee", [64, 128], F32R, kind="ExternalInput").ap()
    out_d = nc.dram_tensor("out", [S, D], BF16, kind="ExternalOutput").ap()

    with tile.TileContext(nc) as tc:
        with tc.tile_pool(name="sb", bufs=1) as sb, \
             tc.tile_pool(name="psS", bufs=2, space="PSUM") as psS, \
             tc.tile_pool(name="psA", bufs=2, space="PSUM") as psA, \
             tc.tile_pool(name="psB", bufs=1, space="PSUM") as psB:

            # ---- persistent SBUF tensors ----
            wo = sb.tile([128, NCT * D], BF16, tag="wo")
            ctab = sb.tile([128, S], BF16, tag="ctab")
            stab = sb.tile([128, S], BF16, tag="stab")
            mask_t = sb.tile([128, 128], BF16, tag="mask")
            ee_t = sb.tile([64, 128], F32R, tag="ee")
            qpair = [sb.tile([128, S], BF16, tag=f"q{p}", name=f"qpair{p}")
                     for p in range(2)]
            kpair = [sb.tile([128, S], BF16, tag=f"k{p}", name=f"kpair{p}")
                     for p in range(2)]
            # V_aug: per seq-tile, per head, a contiguous 65-col block:
            # [64 V dims][1.0]  -> lhsT slice = one block (M=65).  fp8e4:
            # the PV matmuls run in DoubleRow mode (2 k-tiles per pass).
            vaug = sb.tile([128, NST * 260], FP8, tag="vaug")
            opair = sb.tile([128, 1024], BF16, tag="opair")
            recip = sb.tile([64, 1024], F32R, tag="recip")
            attn0 = sb.tile([128, S], BF16, tag="attn0")
            attn1 = sb.tile([128, S], BF16, tag="attn1")

            # ---- input DMAs: critical path (wq0, xt) on sync queue; the
            # rest spread over vector + scalar queues so HWDGE/DMA overlap.
            sbf_cm = tc.tile_pool(name="front", bufs=1)
            sbf = sbf_cm.__enter__()
            # xt is q-chunk-major: [qc (4)][kt (8)][512] so chunk qc is one
            # contiguous DMA and front GEMM of chunk qc only needs chunk qc.
            xt = sbf.tile([128, NQC * NKT * 512], BF16, tag="xt")
            wq0 = sbf.tile([128, NKT * 128], BF16, tag="wq0")
            wq1 = sbf.tile([128, NKT * 128], BF16, tag="wq1")
            wdkv = sbf.tile([128, NKT * DC], BF16, tag="wdkv")
            wupk = sbf.tile([128, NCT * GD], BF16, tag="wupk")
            wupv = sbf.tile([128, NCT * GD], BF16, tag="wupv")
            cpair = [sbf.tile([128, S], BF16, tag=f"c{p}", name=f"cpair{p}")
                     for p in range(2)]
            ry = sbf.tile([128, S], BF16, tag="ry")

            nc.sync.dma_start(
                wq0[:].rearrange("p (t m) -> p t m", t=NKT),
                wq_d[:, 0:128].rearrange("(t p) m -> p t m", p=128))
            for qc in range(NQC):
                nc.sync.dma_start(
                    xt[:, qc * 4096:(qc + 1) * 4096].rearrange(
                        "p (t s) -> p t s", t=NKT),
                    xt_d[:, qc * 512:(qc + 1) * 512].rearrange(
                        "(t p) s -> p t s", p=128))
            nc.scalar.dma_start(
                wdkv[:].rearrange("p (t m) -> p t m", t=NKT),
                wdkv_d.rearrange("(t p) m -> p t m", p=128))
            nc.scalar.dma_start(ctab[:], ctab_d)
            nc.scalar.dma_start(stab[:], stab_d)
            nc.scalar.dma_start(
                wupk[:].rearrange("p (t m) -> p t m", t=NCT),
                wupk_d.rearrange("(t p) m -> p t m", p=128))
            nc.scalar.dma_start(
                wupv[:].rearrange("p (t m) -> p t m", t=NCT),
                wupv_d.rearrange("(t p) m -> p t m", p=128))
            nc.scalar.dma_start(
                wq1[:].rearrange("p (t m) -> p t m", t=NKT),
                wq_d[:, 128:256].rearrange("(t p) m -> p t m", p=128))
            nc.scalar.dma_start(mask_t[:], mask_d)
            nc.scalar.dma_start(ee_t[:], ee_d)
            nc.scalar.dma_start(
                wo[:].rearrange("p (t n) -> p t n", t=NCT),
                wo_d.rearrange("(t p) n -> p t n", p=128))
            nc.gpsimd.memset(
                vaug[:].rearrange("p (t c) -> p t c", c=65)[:, :, 64:65], 1.0)
            nc.gpsimd.memset(recip[:].bitcast(F32), 0.0)

            sbp_cm = tc.tile_pool(name="pt", bufs=5)
            sbp = sbp_cm.__enter__()
            sbo_cm = tc.tile_pool(name="ostagep", bufs=2)
            sbo = sbo_cm.__enter__()

            def front_gemm_qc(w_all, w_off, dst, tag):
                # q-chunk-outer: pipelines with the per-chunk xt DMAs.
                for qc in range(NQC):
                    acc = psA.tile([128, 512], F32, tag="fa",
                                   name=f"f_{tag}_{qc}")
                    for kt in range(NKT):
                        nc.tensor.matmul(
                            acc[:],
                            w_all[:, kt * 128 + w_off:kt * 128 + w_off + 128],
                            xt[:, qc * 4096 + kt * 512:
                               qc * 4096 + (kt + 1) * 512],
                            start=(kt == 0), stop=(kt == NKT - 1),
                        )
                    nc.vector.tensor_copy(dst[:, qc * 512:(qc + 1) * 512],
                                          acc[:])

            def front_gemm_kt(w_all, w_stride, w_off, dst, tag, nkt=NKT,
                              rhs=None):
                # kt-outer with 4 live accumulators: one Ldweights per kt.
                accs = [psS.tile([128, 1024], F32, tag="st",
                                 name=f"fk_{tag}_{a}") for a in range(2)]
                for kt in range(nkt):
                    for qc in range(NQC):
                        r = (xt[:, qc * 4096 + kt * 512:
                                qc * 4096 + (kt + 1) * 512] if rhs is None
                             else rhs[kt][:, qc * 512:(qc + 1) * 512])
                        nc.tensor.matmul(
                            accs[qc // 2][:, (qc % 2) * 512:
                                          (qc % 2 + 1) * 512],
                            w_all[:, kt * w_stride + w_off:
                                  kt * w_stride + w_off + 128],
                            r,
                            start=(kt == 0), stop=(kt == nkt - 1),
                            skip_group_check=True,
                        )
                for qc in range(NQC):
                    nc.vector.tensor_copy(
                        dst[:, qc * 512:(qc + 1) * 512],
                        accs[qc // 2][:, (qc % 2) * 512:(qc % 2 + 1) * 512])

            def front_gemm_kt2(w_all, w_stride, w_off, dst, tag, nkt=NKT,
                               rhs=None):
                # kt-outer, 2 accumulators x 2 passes (psA): for use inside
                # the attention interleave where psS holds score tiles.
                for qh in range(2):
                    accs = [psA.tile([128, 512], F32, tag="fa",
                                     name=f"fk2_{tag}_{qh}_{a}")
                            for a in range(2)]
                    for kt in range(nkt):
                        for a in range(2):
                            qc = qh * 2 + a
                            r = (xt[:, qc * 4096 + kt * 512:
                                    qc * 4096 + (kt + 1) * 512] if rhs is None
                                 else rhs[kt][:, qc * 512:(qc + 1) * 512])
                            nc.tensor.matmul(
                                accs[a][:],
                                w_all[:, kt * w_stride + w_off:
                                      kt * w_stride + w_off + 128],
                                r,
                                start=(kt == 0), stop=(kt == nkt - 1),
                                skip_group_check=True,
                            )
                    for a in range(2):
                        qc = qh * 2 + a
                        nc.vector.tensor_copy(
                            dst[:, qc * 512:(qc + 1) * 512], accs[a][:])

            def rope(t, idx):
                # 32-partition block swap via SBUF->SBUF DMA (whole rows,
                # spread over both HWDGE queues), then chunked bf16 DVE ops
                # (2x mode) so dependent score matmuls unblock per chunk.
                for blk in range(4):
                    nc.sync.dma_start(
                        ry[blk * 32:(blk + 1) * 32, :],
                        t[(blk ^ 1) * 32:(blk ^ 1) * 32 + 32, :])
                for ch in range(4):
                    c0, c1 = ch * 512, (ch + 1) * 512
                    nc.gpsimd.tensor_tensor(ry[:, c0:c1], ry[:, c0:c1],
                                            stab[:, c0:c1],
                                            mybir.AluOpType.mult)
                    nc.vector.tensor_mul(t[:, c0:c1], t[:, c0:c1],
                                         ctab[:, c0:c1])
                    nc.vector.tensor_add(t[:, c0:c1], t[:, c0:c1],
                                         ry[:, c0:c1])

            # ---- front: Q pair0 (DMA-pipelined), latent c, K pair0 ----
            front_gemm_qc(wq0, 0, qpair[0], "q0")
            rope(qpair[0], 0)
            front_gemm_kt(wdkv, DC, 0, cpair[0], "c0")
            front_gemm_kt(wdkv, DC, 128, cpair[1], "c1")
            front_gemm_kt(wupk, GD, 0, kpair[0], "k0", nkt=NCT, rhs=cpair)
            rope(kpair[0], 2)
            # PE fill while rope(k0) completes on DMA+DVE: all V tiles and
            # the pair-1 Q front (in-order PE stream keeps the array busy).

            def vaug_tile(st):
                # V tile: (s,d) natural layout + ones column, bf16.  Runs in
                # the pre-attention window where ACT is idle, so the PSUM ->
                # SBUF copy goes to ACT (GPSIMD cannot read PSUM).
                acc = psA.tile([128, 512], F32, tag="fa", name=f"vacc{st}")
                for kt in range(NCT):
                    nc.tensor.matmul(
                        acc[:, 0:GD],
                        cpair[kt][:, st * 128:(st + 1) * 128],
                        wupv[:, kt * GD:(kt + 1) * GD],
                        start=(kt == 0), stop=(kt == NCT - 1),
                    )
                nc.scalar.copy(
                    vaug[:, st * 260:(st + 1) * 260].rearrange(
                        "p (h c) -> p h c", c=65)[:, :, 0:64],
                    acc[:, 0:GD].rearrange("p (h c) -> p h c", c=64))

            for st in range(NST):
                vaug_tile(st)
            front_gemm_kt2(wq1, 128, 0, qpair[1], "q1")
            rope(qpair[1], 1)

            def attn_pair(pair, attn_t, mid=None):
                qh = qpair[pair]
                kh = kpair[pair]
                for c in range(2):
                    if c == 1 and mid is not None:
                        mid()
                    c0, c1 = 1024 * c, 1024 * (c + 1)
                    for sub in range(2):
                        h = 2 * pair + sub
                        po = 64 * sub
                        # bank bb of ot finishes its causal accumulation at
                        # i == 4*bank+3; its reciprocal + opair copy are
                        # emitted right there so they overlap the other
                        # bank's PV matmuls instead of serializing at the
                        # (c, sub) boundary.
                        ot = psB.tile([128, 1024], F32, tag="ot",
                                      name=f"ot{h}_{c}")
                        vsl = vaug[:].rearrange("p (s c) -> p s c", c=260)
                        for j in range(NST // 2):
                            i0, i1 = 2 * j, 2 * j + 1
                            if c1 <= 128 * i0:
                                continue
                            # pt holds exp(scores) for the k-tile pair
                            # (i0, i1) side by side; the PV matmul runs in
                            # fp8 DoubleRow mode over both tiles at once.
                            pt = sbp.tile([128, 2048], FP8, tag="pt",
                                          name=f"pt{h}_{j}_{c}")
                            for ip, i in enumerate((i0, i1)):
                                qlo = 128 * i
                                if c1 <= qlo:
                                    continue
                                lo = max(qlo, c0)
                                st_t = psS.tile([128, 1024], F32, tag="st",
                                                name=f"stt{h}_{i}_{c}")
                                for bb in range(2):
                                    b0 = c0 + 512 * bb
                                    b1 = b0 + 512
                                    blo = max(lo, b0)
                                    if blo >= b1:
                                        continue
                                    nc.tensor.matmul(
                                        st_t[:, blo - c0:b1 - c0],
                                        kh[po:po + 64, i * 128:(i + 1) * 128],
                                        qh[po:po + 64, blo:b1],
                                        start=True, stop=True,
                                    )
                                if c0 <= qlo < c1:
                                    # additive -inf causal mask on the scores
                                    # (pre-exp) so pt can stay fp8
                                    nc.vector.tensor_add(
                                        st_t[:, qlo - c0:qlo - c0 + 128],
                                        st_t[:, qlo - c0:qlo - c0 + 128],
                                        mask_t[:])
                                nc.scalar.activation(
                                    pt[:, ip * 1024 + lo - c0:
                                       (ip + 1) * 1024],
                                    st_t[:, lo - c0:1024],
                                    mybir.ActivationFunctionType.Exp,
                                    scale=float(1.0 / np.sqrt(DH)),
                                )
                            # strip: columns where only tile i0 contributes
                            slo, shi = max(128 * i0, c0), min(128 * i1, c1)
                            if slo < shi:
                                nc.tensor.matmul(
                                    ot[0:65, slo - c0:shi - c0],
                                    vsl[:, i0, 65 * h:65 * h + 65],
                                    pt[:, slo - c0:shi - c0],
                                    start=(j == 0), stop=False,
                                    skip_group_check=True,
                                )
                            plo = max(128 * i1, c0)
                            ptp = pt.rearrange("p (t n) -> p t n", t=2)
                            for bb in range(2):
                                b0 = c0 + 512 * bb
                                b1 = b0 + 512
                                blo = max(plo, b0)
                                if blo >= b1:
                                    continue
                                nc.tensor.matmul(
                                    ot[0:65, blo - c0:b1 - c0],
                                    vsl[:, i0:i0 + 2, 65 * h:65 * h + 65],
                                    ptp[:, :, blo - c0:b1 - c0],
                                    start=(j == 0),
                                    stop=(j == 4 * c + 2 * bb + 1),
                                    perf_mode=mybir.MatmulPerfMode.DoubleRow,
                                    skip_group_check=True,
                                )
                            if j == 4 * c + 1 or j == 4 * c + 3:
                                bb = (j // 2) % 2
                                with nc.allow_low_precision(
                                        reason="recip to f32r"):
                                    nc.vector.reciprocal(
                                        recip[32 * sub:32 * sub + 1,
                                              bb * 512:(bb + 1) * 512],
                                        ot[64:65, bb * 512:(bb + 1) * 512])
                                nc.vector.tensor_copy(
                                    opair[po:po + 64,
                                          bb * 512:(bb + 1) * 512],
                                    ot[0:64, bb * 512:(bb + 1) * 512])
                    for qc in range(2):
                        rt = psA.tile([128, 512], F32, tag="fa",
                                      name=f"rtt{pair}_{c}_{qc}")
                        nc.tensor.matmul(
                            rt[:], ee_t[:],
                            recip[:, qc * 512:(qc + 1) * 512],
                            start=True, stop=True)
                        nc.vector.tensor_mul(
                            attn_t[:, c0 + qc * 512:c0 + (qc + 1) * 512],
                            opair[:, qc * 512:(qc + 1) * 512], rt[:])

            def wo_gemm(qg_lo, qg_hi, act_ok=False):
                # combined: out = attn0^T @ W_O[0:128] + attn1^T @ W_O[128:]
                for qg in range(qg_lo, qg_hi):
                    ostage = sbo.tile([128, 2 * D], BF16, tag="ostage",
                                      name=f"ostage{qg}")
                    for qi in range(2):
                        qt = qg * 2 + qi
                        accs = [psA.tile([128, 512], F32, tag="fa",
                                         name=f"woacc{qt}_{n}")
                                for n in range(2)]
                        for p in range(2):
                            at = attn0 if p == 0 else attn1
                            for nch in range(2):
                                nc.tensor.matmul(
                                    accs[nch][:],
                                    at[:, qt * 128:(qt + 1) * 128],
                                    wo[:, p * D + nch * 512:
                                       p * D + (nch + 1) * 512],
                                    start=(p == 0), stop=(p == 1),
                                    skip_group_check=True,
                                )
                        for nch in range(2):
                            if nch == 1 and act_ok:
                                nc.scalar.copy(
                                    ostage[:, qi * D + 512:qi * D + 1024],
                                    accs[1][:])
                            else:
                                nc.vector.tensor_copy(
                                    ostage[:, qi * D + nch * 512:
                                           qi * D + (nch + 1) * 512],
                                    accs[nch][:])
                    nc.sync.dma_start(
                        out_d[qg * 256:(qg + 1) * 256, :].rearrange(
                            "(t p) n -> p t n", p=128),
                        ostage[:].rearrange("p (t n) -> p t n", t=2))

            def mid0():
                front_gemm_kt2(wupk, GD, 128, kpair[1], "k1", nkt=NCT,
                               rhs=cpair)
                rope(kpair[1], 3)

            attn_pair(0, attn0, mid=mid0)
            attn_pair(1, attn1, mid=lambda: wo_gemm(0, 4))
            wo_gemm(4, 8, act_ok=True)

            sbo_cm.__exit__(None, None, None)
            sbp_cm.__exit__(None, None, None)
            sbf_cm.__exit__(None, None, None)

    nc.compile()
    return nc


_NC_CACHE = []


def _get_nc():
    if not _NC_CACHE:
        _NC_CACHE.append(_build_nc())
    return _NC_CACHE[0]


def _host_tables():
    theta = 10000.0 ** (-np.arange(0, DH, 2, dtype=np.float64) / DH)  # (32,)
    pos = np.arange(S, dtype=np.float64)
    ang = np.outer(theta, pos)  # (32, S)
    cos = np.cos(ang).astype(np.float32)
    sin = np.sin(ang).astype(np.float32)
    ctab = np.tile(cos, (4, 1))  # (128, S)
    stab = np.concatenate([-sin, sin, -sin, sin], axis=0).astype(np.float32)
    # additive causal mask on scores: 0 where q >= k, else -1e30 (the exp
    # then yields exactly 0, and pt can be stored in fp8)
    mask = np.where(np.arange(128)[None, :] >= np.arange(128)[:, None],
                    0.0, -1e30).astype(ml_dtypes.bfloat16)  # (128,128)
    # rt broadcast matrix: row 0 -> sub0 recip (partitions 0:64 of rt),
    # row 32 -> sub1 recip (partitions 64:128); rows must be 32-aligned.
    ee = np.zeros((64, 128), np.float32)
    ee[0, 0:64] = 1.0
    ee[32, 64:128] = 1.0
    return ctab, stab, mask, ee


def _make_in_maps(X, W_Q, W_down_kv, W_up_K, W_up_V, W_O):
    X = np.asarray(X, np.float32)
    W_Q = np.asarray(W_Q, np.float32)
    W_down_kv = np.asarray(W_down_kv, np.float32)
    W_up_K = np.asarray(W_up_K, np.float32)
    W_up_V = np.asarray(W_up_V, np.float32)
    W_O = np.asarray(W_O, np.float32)

    ctab, stab, mask, ee = _host_tables()
    bf = ml_dtypes.bfloat16
    ctab, stab = ctab.astype(bf), stab.astype(bf)
    # per-head even/odd de-interleave permutation (applied to W_Q and W_up_K
    # output columns; Q.K dot products are invariant under the shared perm)
    perm_head = np.concatenate([np.arange(0, DH, 2), np.arange(1, DH, 2)])

    xts = [np.ascontiguousarray(X[b].T).astype(bf) for b in range(B)]
    in_maps = []
    for c in range(N_CORES):
        b, g = c // 4, c % 4
        cols = np.concatenate(
            [(g * HPC + h) * DH + perm_head for h in range(HPC)])
        vcols = slice(g * GD, (g + 1) * GD)
        in_maps.append({
            "xt": xts[b],
            "wq": np.ascontiguousarray(W_Q[:, cols]).astype(bf),
            "wdkv": W_down_kv.astype(bf),
            "wupk": np.ascontiguousarray(W_up_K[:, cols]).astype(bf),
            "wupv": np.ascontiguousarray(W_up_V[:, vcols]).astype(bf),
            "wo": np.ascontiguousarray(W_O[vcols, :]).astype(bf),
            "ctab": ctab, "stab": stab, "mask": mask, "ee": ee,
        })

    return in_maps


def _gather(res):
    out = np.zeros((B, S, D), np.float32)
    for c in range(N_CORES):
        out[c // 4] += res.results[c]["out"].astype(np.float32)
    return out


def kernel(X, W_Q, W_down_kv, W_up_K, W_up_V, W_O):
    in_maps = _make_in_maps(X, W_Q, W_down_kv, W_up_K, W_up_V, W_O)
    nc = _get_nc()
    res = run_bass_kernel_spmd(nc, in_maps, core_ids=list(range(N_CORES)))
    return _gather(res)


# revision 53
# speedup vs baseline: 1.0349x; 1.0349x over previous
"""MLA-v2 (multi-head latent attention) forward pass on 8 Trainium2 NeuronCores.

Sharding: core c -> (batch b = c // 4, head-group g = c % 4, 4 heads each).
Data parallel over batch; tensor parallel over heads (W_Q / W_up_K / W_up_V
column-sharded, W_O row-sharded).  The compressed latent c_kv is computed
replicated per core.  Each core emits ONE (S, D) bf16 partial (both head
pairs accumulated on-chip in PSUM); the host sums the 4 partials per batch.

On-chip layout is fully "transposed" (feature dim on partitions, sequence on
the free axis) so that no activation transposes are ever needed:
  Q^T = W_Q^T @ X^T      (PSUM -> SBUF, RoPE applied in-place)
  c^T = W_dkv^T @ X^T
  K^T = W_upK^T @ c^T    (RoPE in-place)
  V   = (c^T slice)^T @ W_upV          -> natural (s, d) tiles (JIT in attn)
  S^T[k, q] = (K^T tile)^T @ Q^T       -> softmax over k via matmul tricks
  O'^T = V_aug^T @ exp(S^T)            -> row 64 of V_aug is ones => denom
  out  = attn0^T @ W_O[0:128] + attn1^T @ W_O[128:256]  (single partial)

All activations and weights are bf16 on chip (PSUM accumulation is f32);
softmax denominators/reciprocals stay f32.  RoPE trick: columns of W_Q /
W_up_K are permuted per head so even/odd pairs become [32 evens | 32 odds]
blocks; dot products are invariant because Q and K share the permutation.
Rotation is three elementwise ops plus a 32-partition-block swap (SBUF->SBUF
DMA).  Softmax skips the max-subtraction: scores are bounded (|s| < ~4).
"""

import numpy as np
import ml_dtypes

import concourse.bass as bass
import concourse.bacc as bacc
import concourse.mybir as mybir
import concourse.tile as tile
from concourse.bass_utils import run_bass_kernel_spmd

F32 = mybir.dt.float32
F32R = mybir.dt.float32r
BF16 = mybir.dt.bfloat16
FP8 = mybir.dt.float8e4

B = 2
S = 2048
D = 1024
H = 16
DH = 64
DC = 256
HPC = 4          # heads per core
GD = HPC * DH    # per-core sharded model dim (256)
N_CORES = 8
NKT = D // 128   # k-tiles over D (8)
NCT = DC // 128  # k-tiles over DC (2)
NST = S // 128   # seq tiles (16)
NQC = S // 512   # 512-wide q chunks (4)


def _build_nc():
    nc = bacc.Bacc("TRN2", target_bir_lowering=False, debug=False,
                   num_devices=N_CORES)

    xt_d = nc.dram_tensor("xt", [D, S], BF16, kind="ExternalInput").ap()
    wq_d = nc.dram_tensor("wq", [D, GD], BF16, kind="ExternalInput").ap()
    wdkv_d = nc.dram_tensor("wdkv", [D, DC], BF16, kind="ExternalInput").ap()
    wupk_d = nc.dram_tensor("wupk", [DC, GD], BF16, kind="ExternalInput").ap()
    wupv_d = nc.dram_tensor("wupv", [DC, GD], BF16, kind="ExternalInput").ap()
    wo_d = nc.dram_tensor("wo", [GD, D], BF16, kind="ExternalInput").ap()
    ctab_d = nc.dram_tensor("ctab", [128, S], BF16, kind="ExternalInput").ap()
    stab_d = nc.dram_tensor("stab", [128, S], BF16, kind="ExternalInput").ap()
    mask_d = nc.dram_tensor("mask", [128, 128], BF16, kind="ExternalInput").ap()
    ee_d = nc.dram_tensor("ee", [64, 128], F32R, kind="ExternalInput").ap()
    out_d = nc.dram_tensor("out", [S, D], BF16, kind="ExternalOutput").ap()

    with tile.TileContext(nc) as tc:
        with tc.tile_pool(name="sb", bufs=1) as sb, \
             tc.tile_pool(name="psS", bufs=2, space="PSUM") as psS, \
             tc.tile_pool(name="psA", bufs=2, space="PSUM") as psA, \
             tc.tile_pool(name="psB", bufs=1, space="PSUM") as psB:

            # ---- persistent SBUF tensors ----
            wo = sb.tile([128, NCT * D], BF16, tag="wo")
            ctab = sb.tile([128, S], BF16, tag="ctab")
            stab = sb.tile([128, S], BF16, tag="stab")
            mask_t = sb.tile([128, 128], BF16, tag="mask")
            ee_t = sb.tile([64, 128], F32R, tag="ee")
            qpair = [sb.tile([128, S], BF16, tag=f"q{p}", name=f"qpair{p}")
                     for p in range(2)]
            kpair = [sb.tile([128, S], BF16, tag=f"k{p}", name=f"kpair{p}")
                     for p in range(2)]
            # V_aug: per seq-tile, per head, a contiguous 65-col block:
            # [64 V dims][1.0]  -> lhsT slice = one block (M=65).  fp8e4:
            # the PV matmuls run in DoubleRow mode (2 k-tiles per pass).
            vaug = sb.tile([128, NST * 260], FP8, tag="vaug")
            opair = sb.tile([128, 1024], BF16, tag="opair")
            recip = sb.tile([64, 1024], F32R, tag="recip")
            attn0 = sb.tile([128, S], BF16, tag="attn0")
            attn1 = sb.tile([128, S], BF16, tag="attn1")

            # ---- input DMAs: critical path (wq0, xt) on sync queue; the
            # rest spread over vector + scalar queues so HWDGE/DMA overlap.
            sbf_cm = tc.tile_pool(name="front", bufs=1)
            sbf = sbf_cm.__enter__()
            # xt is q-chunk-major: [qc (4)][kt (8)][512] so chunk qc is one
            # contiguous DMA and front GEMM of chunk qc only needs chunk qc.
            xt = sbf.tile([128, NQC * NKT * 512], BF16, tag="xt")
            wq0 = sbf.tile([128, NKT * 128], BF16, tag="wq0")
            wq1 = sbf.tile([128, NKT * 128], BF16, tag="wq1")
            wdkv = sbf.tile([128, NKT * DC], BF16, tag="wdkv")
            wupk = sbf.tile([128, NCT * GD], BF16, tag="wupk")
            wupv = sbf.tile([128, NCT * GD], BF16, tag="wupv")
            cpair = [sbf.tile([128, S], BF16, tag=f"c{p}", name=f"cpair{p}")
                     for p in range(2)]
            ry = sbf.tile([128, S], BF16, tag="ry")
            ry2 = sbf.tile([128, S], BF16, tag="ry2")

            nc.sync.dma_start(
                wq0[:].rearrange("p (t m) -> p t m", t=NKT),
                wq_d[:, 0:128].rearrange("(t p) m -> p t m", p=128))
            for qc in range(NQC):
                nc.sync.dma_start(
                    xt[:, qc * 4096:(qc + 1) * 4096].rearrange(
                        "p (t s) -> p t s", t=NKT),
                    xt_d[:, qc * 512:(qc + 1) * 512].rearrange(
                        "(t p) s -> p t s", p=128))
            nc.scalar.dma_start(
                wdkv[:].rearrange("p (t m) -> p t m", t=NKT),
                wdkv_d.rearrange("(t p) m -> p t m", p=128))
            nc.scalar.dma_start(ctab[:], ctab_d)
            nc.scalar.dma_start(stab[:], stab_d)
            nc.scalar.dma_start(
                wupk[:].rearrange("p (t m) -> p t m", t=NCT),
                wupk_d.rearrange("(t p) m -> p t m", p=128))
            nc.scalar.dma_start(
                wupv[:].rearrange("p (t m) -> p t m", t=NCT),
                wupv_d.rearrange("(t p) m -> p t m", p=128))
            nc.scalar.dma_start(
                wq1[:].rearrange("p (t m) -> p t m", t=NKT),
                wq_d[:, 128:256].rearrange("(t p) m -> p t m", p=128))
            nc.scalar.dma_start(mask_t[:], mask_d)
            nc.scalar.dma_start(ee_t[:], ee_d)
            nc.scalar.dma_start(
                wo[:].rearrange("p (t n) -> p t n", t=NCT),
                wo_d.rearrange("(t p) n -> p t n", p=128))
            nc.gpsimd.memset(
                vaug[:].rearrange("p (t c) -> p t c", c=65)[:, :, 64:65], 1.0)
            nc.gpsimd.memset(recip[:].bitcast(F32), 0.0)

            sbp_cm = tc.tile_pool(name="pt", bufs=5)
            sbp = sbp_cm.__enter__()
            sbo_cm = tc.tile_pool(name="ostagep", bufs=2)
            sbo = sbo_cm.__enter__()

            def front_gemm_qc(w_all, w_off, dst, tag):
                # q-chunk-outer: pipelines with the per-chunk xt DMAs.
                for qc in range(NQC):
                    acc = psA.tile([128, 512], F32, tag="fa",
                                   name=f"f_{tag}_{qc}")
                    for kt in range(NKT):
                        nc.tensor.matmul(
                            acc[:],
                            w_all[:, kt * 128 + w_off:kt * 128 + w_off + 128],
                            xt[:, qc * 4096 + kt * 512:
                               qc * 4096 + (kt + 1) * 512],
                            start=(kt == 0), stop=(kt == NKT - 1),
                        )
                    nc.vector.tensor_copy(dst[:, qc * 512:(qc + 1) * 512],
                                          acc[:])

            def front_gemm_kt(w_all, w_stride, w_off, dst, tag, nkt=NKT,
                              rhs=None):
                # kt-outer with 4 live accumulators: one Ldweights per kt.
                accs = [psS.tile([128, 1024], F32, tag="st",
                                 name=f"fk_{tag}_{a}") for a in range(2)]
                for kt in range(nkt):
                    for qc in range(NQC):
                        r = (xt[:, qc * 4096 + kt * 512:
                                qc * 4096 + (kt + 1) * 512] if rhs is None
                             else rhs[kt][:, qc * 512:(qc + 1) * 512])
                        nc.tensor.matmul(
                            accs[qc // 2][:, (qc % 2) * 512:
                                          (qc % 2 + 1) * 512],
                            w_all[:, kt * w_stride + w_off:
                                  kt * w_stride + w_off + 128],
                            r,
                            start=(kt == 0), stop=(kt == nkt - 1),
                            skip_group_check=True,
                        )
                for qc in range(NQC):
                    nc.vector.tensor_copy(
                        dst[:, qc * 512:(qc + 1) * 512],
                        accs[qc // 2][:, (qc % 2) * 512:(qc % 2 + 1) * 512])

            def front_gemm_kt2(w_all, w_stride, w_off, dst, tag, nkt=NKT,
                               rhs=None):
                # kt-outer, 2 accumulators x 2 passes (psA): for use inside
                # the attention interleave where psS holds score tiles.
                for qh in range(2):
                    accs = [psA.tile([128, 512], F32, tag="fa",
                                     name=f"fk2_{tag}_{qh}_{a}")
                            for a in range(2)]
                    for kt in range(nkt):
                        for a in range(2):
                            qc = qh * 2 + a
                            r = (xt[:, qc * 4096 + kt * 512:
                                    qc * 4096 + (kt + 1) * 512] if rhs is None
                                 else rhs[kt][:, qc * 512:(qc + 1) * 512])
                            nc.tensor.matmul(
                                accs[a][:],
                                w_all[:, kt * w_stride + w_off:
                                      kt * w_stride + w_off + 128],
                                r,
                                start=(kt == 0), stop=(kt == nkt - 1),
                                skip_group_check=True,
                            )
                    for a in range(2):
                        qc = qh * 2 + a
                        nc.vector.tensor_copy(
                            dst[:, qc * 512:(qc + 1) * 512], accs[a][:])

            def rope_swaps(t, yb):
                # 32-partition block swap via SBUF->SBUF DMA (whole rows)
                for blk in range(4):
                    nc.sync.dma_start(
                        yb[blk * 32:(blk + 1) * 32, :],
                        t[(blk ^ 1) * 32:(blk ^ 1) * 32 + 32, :])

            def rope_chunk(t, yb, ch, pool_only=False):
                c0, c1 = ch * 512, (ch + 1) * 512
                nc.gpsimd.tensor_tensor(yb[:, c0:c1], yb[:, c0:c1],
                                        stab[:, c0:c1],
                                        mybir.AluOpType.mult)
                if pool_only:
                    # keep DVE free for attention mask adds: the whole
                    # rotation runs on the (otherwise idle) Pool engine
                    nc.gpsimd.tensor_tensor(t[:, c0:c1], t[:, c0:c1],
                                            ctab[:, c0:c1],
                                            mybir.AluOpType.mult)
                    nc.gpsimd.tensor_tensor(t[:, c0:c1], t[:, c0:c1],
                                            yb[:, c0:c1],
                                            mybir.AluOpType.add)
                else:
                    nc.vector.tensor_mul(t[:, c0:c1], t[:, c0:c1],
                                         ctab[:, c0:c1])
                    nc.vector.tensor_add(t[:, c0:c1], t[:, c0:c1],
                                         yb[:, c0:c1])

            def rope(t, idx):
                rope_swaps(t, ry)
                for ch in range(4):
                    rope_chunk(t, ry, ch)

            # ---- front: Q pair0 (DMA-pipelined), latent c, K pair0 ----
            front_gemm_qc(wq0, 0, qpair[0], "q0")
            rope(qpair[0], 0)
            front_gemm_kt(wdkv, DC, 0, cpair[0], "c0")
            front_gemm_kt(wdkv, DC, 128, cpair[1], "c1")
            front_gemm_kt(wupk, GD, 0, kpair[0], "k0", nkt=NCT, rhs=cpair)
            rope(kpair[0], 2)
            # PE fill while rope(k0) completes on DMA+DVE: all V tiles and
            # the pair-1 Q front (in-order PE stream keeps the array busy).

            def vaug_tile(st):
                # V tile: (s,d) natural layout + ones column, bf16.  Runs in
                # the pre-attention window where ACT is idle, so the PSUM ->
                # SBUF copy goes to ACT (GPSIMD cannot read PSUM).
                acc = psA.tile([128, 512], F32, tag="fa", name=f"vacc{st}")
                for kt in range(NCT):
                    nc.tensor.matmul(
                        acc[:, 0:GD],
                        cpair[kt][:, st * 128:(st + 1) * 128],
                        wupv[:, kt * GD:(kt + 1) * GD],
                        start=(kt == 0), stop=(kt == NCT - 1),
                    )
                nc.scalar.copy(
                    vaug[:, st * 260:(st + 1) * 260].rearrange(
                        "p (h c) -> p h c", c=65)[:, :, 0:64],
                    acc[:, 0:GD].rearrange("p (h c) -> p h c", c=64))

            for st in range(NST):
                vaug_tile(st)
            front_gemm_kt2(wq1, 128, 0, qpair[1], "q1")
            rope_swaps(qpair[1], ry2)

            def attn_pair(pair, attn_t, mid=None, subfill=None, post_qc=None):
                qh = qpair[pair]
                kh = kpair[pair]
                for c in range(2):
                    if c == 1 and mid is not None:
                        mid()
                    c0, c1 = 1024 * c, 1024 * (c + 1)
                    for sub in range(2):
                        h = 2 * pair + sub
                        po = 64 * sub
                        # bank bb of ot finishes its causal accumulation at
                        # i == 4*bank+3; its reciprocal + opair copy are
                        # emitted right there so they overlap the other
                        # bank's PV matmuls instead of serializing at the
                        # (c, sub) boundary.
                        ot = psB.tile([128, 1024], F32, tag="ot",
                                      name=f"ot{h}_{c}")
                        vsl = vaug[:].rearrange("p (s c) -> p s c", c=260)
                        for j in range(NST // 2):
                            i0, i1 = 2 * j, 2 * j + 1
                            if c1 <= 128 * i0:
                                continue
                            # pt holds exp(scores) for the k-tile pair
                            # (i0, i1) side by side; the PV matmul runs in
                            # fp8 DoubleRow mode over both tiles at once.
                            pt = sbp.tile([128, 2048], FP8, tag="pt",
                                          name=f"pt{h}_{j}_{c}")
                            for ip, i in enumerate((i0, i1)):
                                qlo = 128 * i
                                if c1 <= qlo:
                                    continue
                                lo = max(qlo, c0)
                                st_t = psS.tile([128, 1024], F32, tag="st",
                                                name=f"stt{h}_{i}_{c}")
                                for bb in range(2):
                                    b0 = c0 + 512 * bb
                                    b1 = b0 + 512
                                    blo = max(lo, b0)
                                    if blo >= b1:
                                        continue
                                    nc.tensor.matmul(
                                        st_t[:, blo - c0:b1 - c0],
                                        kh[po:po + 64, i * 128:(i + 1) * 128],
                                        qh[po:po + 64, blo:b1],
                                        start=True, stop=True,
                                    )
                                if c0 <= qlo < c1:
                                    # additive -inf causal mask on the scores
                                    # (pre-exp) so pt can stay fp8
                                    nc.vector.tensor_add(
                                        st_t[:, qlo - c0:qlo - c0 + 128],
                                        st_t[:, qlo - c0:qlo - c0 + 128],
                                        mask_t[:])
                                nc.scalar.activation(
                                    pt[:, ip * 1024 + lo - c0:
                                       (ip + 1) * 1024],
                                    st_t[:, lo - c0:1024],
                                    mybir.ActivationFunctionType.Exp,
                                    scale=float(1.0 / np.sqrt(DH)),
                                )
                            # strip: columns where only tile i0 contributes
                            slo, shi = max(128 * i0, c0), min(128 * i1, c1)
                            if slo < shi:
                                nc.tensor.matmul(
                                    ot[0:65, slo - c0:shi - c0],
                                    vsl[:, i0, 65 * h:65 * h + 65],
                                    pt[:, slo - c0:shi - c0],
                                    start=(j == 0), stop=False,
                                    skip_group_check=True,
                                )
                            plo = max(128 * i1, c0)
                            ptp = pt.rearrange("p (t n) -> p t n", t=2)
                            for bb in range(2):
                                b0 = c0 + 512 * bb
                                b1 = b0 + 512
                                blo = max(plo, b0)
                                if blo >= b1:
                                    continue
                                nc.tensor.matmul(
                                    ot[0:65, blo - c0:b1 - c0],
                                    vsl[:, i0:i0 + 2, 65 * h:65 * h + 65],
                                    ptp[:, :, blo - c0:b1 - c0],
                                    start=(j == 0),
                                    stop=(j == 4 * c + 2 * bb + 1),
                                    perf_mode=mybir.MatmulPerfMode.DoubleRow,
                                    skip_group_check=True,
                                )
                            if j == 4 * c + 1 or j == 4 * c + 3:
                                bb = (j // 2) % 2
                                with nc.allow_low_precision(
                                        reason="recip to f32r"):
                                    nc.vector.reciprocal(
                                        recip[32 * sub:32 * sub + 1,
                                              bb * 512:(bb + 1) * 512],
                                        ot[64:65, bb * 512:(bb + 1) * 512])
                                nc.vector.tensor_copy(
                                    opair[po:po + 64,
                                          bb * 512:(bb + 1) * 512],
                                    ot[0:64, bb * 512:(bb + 1) * 512])
                        if subfill is not None:
                            subfill(c, sub)
                    for qc in range(2):
                        rt = psA.tile([128, 512], F32, tag="fa",
                                      name=f"rtt{pair}_{c}_{qc}")
                        nc.tensor.matmul(
                            rt[:], ee_t[:],
                            recip[:, qc * 512:(qc + 1) * 512],
                            start=True, stop=True)
                        nc.vector.tensor_mul(
                            attn_t[:, c0 + qc * 512:c0 + (qc + 1) * 512],
                            opair[:, qc * 512:(qc + 1) * 512], rt[:])
                        if post_qc is not None:
                            post_qc(c, qc)

            def wo_gemm(qg_lo, qg_hi, act_ok=False):
                # combined: out = attn0^T @ W_O[0:128] + attn1^T @ W_O[128:]
                for qg in range(qg_lo, qg_hi):
                    ostage = sbo.tile([128, 2 * D], BF16, tag="ostage",
                                      name=f"ostage{qg}")
                    for qi in range(2):
                        qt = qg * 2 + qi
                        accs = [psA.tile([128, 512], F32, tag="fa",
                                         name=f"woacc{qt}_{n}")
                                for n in range(2)]
                        for p in range(2):
                            at = attn0 if p == 0 else attn1
                            for nch in range(2):
                                nc.tensor.matmul(
                                    accs[nch][:],
                                    at[:, qt * 128:(qt + 1) * 128],
                                    wo[:, p * D + nch * 512:
                                       p * D + (nch + 1) * 512],
                                    start=(p == 0), stop=(p == 1),
                                    skip_group_check=True,
                                )
                        for nch in range(2):
                            if nch == 1 and act_ok:
                                nc.scalar.copy(
                                    ostage[:, qi * D + 512:qi * D + 1024],
                                    accs[1][:])
                            else:
                                nc.vector.tensor_copy(
                                    ostage[:, qi * D + nch * 512:
                                           qi * D + (nch + 1) * 512],
                                    accs[nch][:])
                    nc.sync.dma_start(
                        out_d[qg * 256:(qg + 1) * 256, :].rearrange(
                            "(t p) n -> p t n", p=128),
                        ostage[:].rearrange("p (t n) -> p t n", t=2))

            def mid0():
                front_gemm_kt2(wupk, GD, 128, kpair[1], "k1", nkt=NCT,
                               rhs=cpair)
                rope_swaps(kpair[1], ry2)
                rope_chunk(kpair[1], ry2, 0, pool_only=True)
                rope_chunk(kpair[1], ry2, 1, pool_only=True)

            def fill0(c, sub):
                # rope q1 / k1 rotations run on Pool between attention sub
                # blocks so they never delay DVE mask adds or ACT exps
                if c == 0:
                    rope_chunk(qpair[1], ry2, 2 * sub, pool_only=True)
                    rope_chunk(qpair[1], ry2, 2 * sub + 1, pool_only=True)
                elif sub == 0:
                    rope_chunk(kpair[1], ry2, 2, pool_only=True)
                    rope_chunk(kpair[1], ry2, 3, pool_only=True)

            attn_pair(0, attn0, mid=mid0, subfill=fill0)

            def post_qc1(c, qc):
                if c == 1 and qc == 0:
                    wo_gemm(4, 6)

            attn_pair(1, attn1, mid=lambda: wo_gemm(0, 4), post_qc=post_qc1)
            wo_gemm(6, 8, act_ok=True)

            sbo_cm.__exit__(None, None, None)
            sbp_cm.__exit__(None, None, None)
            sbf_cm.__exit__(None, None, None)

    nc.compile()
    return nc


_NC_CACHE = []


def _get_nc():
    if not _NC_CACHE:
        _NC_CACHE.append(_build_nc())
    return _NC_CACHE[0]


def _host_tables():
    theta = 10000.0 ** (-np.arange(0, DH, 2, dtype=np.float64) / DH)  # (32,)
    pos = np.arange(S, dtype=np.float64)
    ang = np.outer(theta, pos)  # (32, S)
    cos = np.cos(ang).astype(np.float32)
    sin = np.sin(ang).astype(np.float32)
    ctab = np.tile(cos, (4, 1))  # (128, S)
    stab = np.concatenate([-sin, sin, -sin, sin], axis=0).astype(np.float32)
    # additive causal mask on scores: 0 where q >= k, else -1e30 (the exp
    # then yields exactly 0, and pt can be stored in fp8)
    mask = np.where(np.arange(128)[None, :] >= np.arange(128)[:, None],
                    0.0, -1e30).astype(ml_dtypes.bfloat16)  # (128,128)
    # rt broadcast matrix: row 0 -> sub0 recip (partitions 0:64 of rt),
    # row 32 -> sub1 recip (partitions 64:128); rows must be 32-aligned.
    ee = np.zeros((64, 128), np.float32)
    ee[0, 0:64] = 1.0
    ee[32, 64:128] = 1.0
    return ctab, stab, mask, ee


def _make_in_maps(X, W_Q, W_down_kv, W_up_K, W_up_V, W_O):
    X = np.asarray(X, np.float32)
    W_Q = np.asarray(W_Q, np.float32)
    W_down_kv = np.asarray(W_down_kv, np.float32)
    W_up_K = np.asarray(W_up_K, np.float32)
    W_up_V = np.asarray(W_up_V, np.float32)
    W_O = np.asarray(W_O, np.float32)

    ctab, stab, mask, ee = _host_tables()
    bf = ml_dtypes.bfloat16
    ctab, stab = ctab.astype(bf), stab.astype(bf)
    # per-head even/odd de-interleave permutation (applied to W_Q and W_up_K
    # output columns; Q.K dot products are invariant under the shared perm)
    perm_head = np.concatenate([np.arange(0, DH, 2), np.arange(1, DH, 2)])

    xts = [np.ascontiguousarray(X[b].T).astype(bf) for b in range(B)]
    in_maps = []
    for c in range(N_CORES):
        b, g = c // 4, c % 4
        cols = np.concatenate(
            [(g * HPC + h) * DH + perm_head for h in range(HPC)])
        vcols = slice(g * GD, (g + 1) * GD)
        in_maps.append({
            "xt": xts[b],
            "wq": np.ascontiguousarray(W_Q[:, cols]).astype(bf),
            "wdkv": W_down_kv.astype(bf),
            "wupk": np.ascontiguousarray(W_up_K[:, cols]).astype(bf),
            "wupv": np.ascontiguousarray(W_up_V[:, vcols]).astype(bf),
            "wo": np.ascontiguousarray(W_O[vcols, :]).astype(bf),
            "ctab": ctab, "stab": stab, "mask": mask, "ee": ee,
        })

    return in_maps


def _gather(res):
    out = np.zeros((B, S, D), np.float32)
    for c in range(N_CORES):
        out[c // 4] += res.results[c]["out"].astype(np.float32)
    return out


def kernel(X, W_Q, W_down_kv, W_up_K, W_up_V, W_O):
    in_maps = _make_in_maps(X, W_Q, W_down_kv, W_up_K, W_up_V, W_O)
    nc = _get_nc()
    res = run_bass_kernel_spmd(nc, in_maps, core_ids=list(range(N_CORES)))
    return _gather(res)


# revision 64
# speedup vs baseline: 1.1722x; 1.1327x over previous
"""MLA-v2 (multi-head latent attention) forward pass on 8 Trainium2 NeuronCores.

Sharding: core c -> (batch b = c // 4, head-group g = c % 4, 4 heads each).
Data parallel over batch; tensor parallel over heads (W_Q / W_up_K / W_up_V
column-sharded, W_O row-sharded).  The compressed latent c_kv is computed
replicated per core.  Each core emits ONE (S, D) bf16 partial (both head
pairs accumulated on-chip in PSUM); the host sums the 4 partials per batch.

On-chip layout is fully "transposed" (feature dim on partitions, sequence on
the free axis) so that no activation transposes are ever needed:
  Q^T = W_Q^T @ X^T      (PSUM -> SBUF, RoPE applied in-place)
  c^T = W_dkv^T @ X^T
  K^T = W_upK^T @ c^T    (RoPE in-place)
  V   = (c^T slice)^T @ W_upV          -> natural (s, d) tiles (JIT in attn)
  S^T[k, q] = (K^T tile)^T @ Q^T       -> softmax over k via matmul tricks
  O'^T = V_aug^T @ exp(S^T)            -> row 64 of V_aug is ones => denom
  out  = attn0^T @ W_O[0:128] + attn1^T @ W_O[128:256]  (single partial)

All activations and weights are bf16 on chip (PSUM accumulation is f32);
softmax denominators/reciprocals stay f32.  RoPE trick: columns of W_Q /
W_up_K are permuted per head so even/odd pairs become [32 evens | 32 odds]
blocks; dot products are invariant because Q and K share the permutation.
Rotation is three elementwise ops plus a 32-partition-block swap (SBUF->SBUF
DMA).  Softmax skips the max-subtraction: scores are bounded (|s| < ~4).
"""

import numpy as np
import ml_dtypes

import concourse.bass as bass
import concourse.bacc as bacc
import concourse.mybir as mybir
import concourse.tile as tile
from concourse.bass_utils import run_bass_kernel_spmd

F32 = mybir.dt.float32
F32R = mybir.dt.float32r
BF16 = mybir.dt.bfloat16
FP8 = mybir.dt.float8e4

B = 2
S = 2048
D = 1024
H = 16
DH = 64
DC = 256
HPC = 4          # heads per core
GD = HPC * DH    # per-core sharded model dim (256)
N_CORES = 8
NKT = D // 128   # k-tiles over D (8)
NCT = DC // 128  # k-tiles over DC (2)
NST = S // 128   # seq tiles (16)
NQC = S // 512   # 512-wide q chunks (4)


def _build_nc():
    nc = bacc.Bacc("TRN2", target_bir_lowering=False, debug=False,
                   num_devices=N_CORES)

    xt_d = nc.dram_tensor("xt", [D, S], BF16, kind="ExternalInput").ap()
    wq_d = nc.dram_tensor("wq", [D, GD], BF16, kind="ExternalInput").ap()
    wdkv_d = nc.dram_tensor("wdkv", [D, DC], BF16, kind="ExternalInput").ap()
    wupk_d = nc.dram_tensor("wupk", [DC, GD], BF16, kind="ExternalInput").ap()
    wupv_d = nc.dram_tensor("wupv", [DC, GD], BF16, kind="ExternalInput").ap()
    wo_d = nc.dram_tensor("wo", [GD, D], BF16, kind="ExternalInput").ap()
    ctab_d = nc.dram_tensor("ctab", [128, S], BF16, kind="ExternalInput").ap()
    stab_d = nc.dram_tensor("stab", [128, S], BF16, kind="ExternalInput").ap()
    mask_d = nc.dram_tensor("mask", [128, 128], BF16, kind="ExternalInput").ap()
    ident_d = nc.dram_tensor("ident", [128, 128], BF16,
                             kind="ExternalInput").ap()
    ee_d = nc.dram_tensor("ee", [64, 128], F32R, kind="ExternalInput").ap()
    out_d = nc.dram_tensor("out", [S, D], BF16, kind="ExternalOutput").ap()

    with tile.TileContext(nc) as tc:
        with tc.tile_pool(name="sb", bufs=1) as sb, \
             tc.tile_pool(name="psS", bufs=2, space="PSUM") as psS, \
             tc.tile_pool(name="psA", bufs=2, space="PSUM") as psA, \
             tc.tile_pool(name="psB", bufs=1, space="PSUM") as psB:

            # ---- persistent SBUF tensors ----
            wo = sb.tile([128, NCT * D], BF16, tag="wo")
            ctab = sb.tile([128, S], BF16, tag="ctab")
            stab = sb.tile([128, S], BF16, tag="stab")
            mask_t = sb.tile([128, 128], BF16, tag="mask")
            ident_t = sb.tile([128, 128], BF16, tag="ident")
            ee_t = sb.tile([64, 128], F32R, tag="ee")
            qpair = [sb.tile([128, S], BF16, tag=f"q{p}", name=f"qpair{p}")
                     for p in range(2)]
            kpair = [sb.tile([128, S], BF16, tag=f"k{p}", name=f"kpair{p}")
                     for p in range(2)]
            # V_aug: per seq-tile, per head, a contiguous 65-col block:
            # [64 V dims][1.0]  -> lhsT slice = one block (M=65).  fp8e4:
            # the PV matmuls run in DoubleRow mode (2 k-tiles per pass).
            vaug = sb.tile([128, NST * 260], FP8, tag="vaug")
            opair = sb.tile([128, 1024], BF16, tag="opair")
            recip = sb.tile([64, 1024], F32R, tag="recip")
            attn0 = sb.tile([128, S], BF16, tag="attn0")
            attn1 = sb.tile([128, S], BF16, tag="attn1")

            # ---- input DMAs: critical path (wq0, xt) on sync queue; the
            # rest spread over vector + scalar queues so HWDGE/DMA overlap.
            sbf_cm = tc.tile_pool(name="front", bufs=1)
            sbf = sbf_cm.__enter__()
            # xt is q-chunk-major: [qc (4)][kt (8)][512] so chunk qc is one
            # contiguous DMA and front GEMM of chunk qc only needs chunk qc.
            xt = sbf.tile([128, NQC * NKT * 512], BF16, tag="xt")
            wq0 = sbf.tile([128, NKT * 128], BF16, tag="wq0")
            wq1 = sbf.tile([128, NKT * 128], BF16, tag="wq1")
            wdkv = sbf.tile([128, NKT * DC], BF16, tag="wdkv")
            wupk = sbf.tile([128, NCT * GD], BF16, tag="wupk")
            wupv = sbf.tile([128, NCT * GD], BF16, tag="wupv")
            cpair = [sbf.tile([128, S], BF16, tag=f"c{p}", name=f"cpair{p}")
                     for p in range(2)]
            ry = sbf.tile([128, S], BF16, tag="ry")
            ry2 = sbf.tile([128, S], BF16, tag="ry2")

            nc.sync.dma_start(
                wq0[:].rearrange("p (t m) -> p t m", t=NKT),
                wq_d[:, 0:128].rearrange("(t p) m -> p t m", p=128))
            for qc in range(NQC):
                nc.sync.dma_start(
                    xt[:, qc * 4096:(qc + 1) * 4096].rearrange(
                        "p (t s) -> p t s", t=NKT),
                    xt_d[:, qc * 512:(qc + 1) * 512].rearrange(
                        "(t p) s -> p t s", p=128))
            nc.scalar.dma_start(
                wdkv[:].rearrange("p (t m) -> p t m", t=NKT),
                wdkv_d.rearrange("(t p) m -> p t m", p=128))
            nc.scalar.dma_start(ctab[:], ctab_d)
            nc.scalar.dma_start(stab[:], stab_d)
            nc.scalar.dma_start(
                wupk[:].rearrange("p (t m) -> p t m", t=NCT),
                wupk_d.rearrange("(t p) m -> p t m", p=128))
            nc.scalar.dma_start(
                wupv[:].rearrange("p (t m) -> p t m", t=NCT),
                wupv_d.rearrange("(t p) m -> p t m", p=128))
            nc.scalar.dma_start(
                wq1[:].rearrange("p (t m) -> p t m", t=NKT),
                wq_d[:, 128:256].rearrange("(t p) m -> p t m", p=128))
            nc.scalar.dma_start(mask_t[:], mask_d)
            nc.scalar.dma_start(ident_t[:], ident_d)
            nc.scalar.dma_start(ee_t[:], ee_d)
            nc.scalar.dma_start(
                wo[:].rearrange("p (t n) -> p t n", t=NCT),
                wo_d.rearrange("(t p) n -> p t n", p=128))
            nc.gpsimd.memset(
                vaug[:].rearrange("p (t c) -> p t c", c=65)[:, :, 64:65], 1.0)
            nc.gpsimd.memset(recip[:].bitcast(F32), 0.0)

            sbp_cm = tc.tile_pool(name="pt", bufs=5)
            sbp = sbp_cm.__enter__()
            sbo_cm = tc.tile_pool(name="ostagep", bufs=2)
            sbo = sbo_cm.__enter__()

            def front_gemm_qc(w_all, w_off, dst, tag):
                # q-chunk-outer: pipelines with the per-chunk xt DMAs.
                for qc in range(NQC):
                    acc = psA.tile([128, 512], F32, tag="fa",
                                   name=f"f_{tag}_{qc}")
                    for kt in range(NKT):
                        nc.tensor.matmul(
                            acc[:],
                            w_all[:, kt * 128 + w_off:kt * 128 + w_off + 128],
                            xt[:, qc * 4096 + kt * 512:
                               qc * 4096 + (kt + 1) * 512],
                            start=(kt == 0), stop=(kt == NKT - 1),
                        )
                    nc.vector.tensor_copy(dst[:, qc * 512:(qc + 1) * 512],
                                          acc[:])

            def front_gemm_kt(w_all, w_stride, w_off, dst, tag, nkt=NKT,
                              rhs=None):
                # kt-outer with 4 live accumulators: one Ldweights per kt.
                accs = [psS.tile([128, 1024], F32, tag="st",
                                 name=f"fk_{tag}_{a}") for a in range(2)]
                for kt in range(nkt):
                    for qc in range(NQC):
                        r = (xt[:, qc * 4096 + kt * 512:
                                qc * 4096 + (kt + 1) * 512] if rhs is None
                             else rhs[kt][:, qc * 512:(qc + 1) * 512])
                        nc.tensor.matmul(
                            accs[qc // 2][:, (qc % 2) * 512:
                                          (qc % 2 + 1) * 512],
                            w_all[:, kt * w_stride + w_off:
                                  kt * w_stride + w_off + 128],
                            r,
                            start=(kt == 0), stop=(kt == nkt - 1),
                            skip_group_check=True,
                        )
                for qc in range(NQC):
                    nc.vector.tensor_copy(
                        dst[:, qc * 512:(qc + 1) * 512],
                        accs[qc // 2][:, (qc % 2) * 512:(qc % 2 + 1) * 512])

            def front_gemm_kt2(w_all, w_stride, w_off, dst, tag, nkt=NKT,
                               rhs=None):
                # kt-outer, 2 accumulators x 2 passes (psA): for use inside
                # the attention interleave where psS holds score tiles.
                for qh in range(2):
                    accs = [psA.tile([128, 512], F32, tag="fa",
                                     name=f"fk2_{tag}_{qh}_{a}")
                            for a in range(2)]
                    for kt in range(nkt):
                        for a in range(2):
                            qc = qh * 2 + a
                            r = (xt[:, qc * 4096 + kt * 512:
                                    qc * 4096 + (kt + 1) * 512] if rhs is None
                                 else rhs[kt][:, qc * 512:(qc + 1) * 512])
                            nc.tensor.matmul(
                                accs[a][:],
                                w_all[:, kt * w_stride + w_off:
                                      kt * w_stride + w_off + 128],
                                r,
                                start=(kt == 0), stop=(kt == nkt - 1),
                                skip_group_check=True,
                            )
                    for a in range(2):
                        qc = qh * 2 + a
                        nc.vector.tensor_copy(
                            dst[:, qc * 512:(qc + 1) * 512], accs[a][:])

            def rope_swaps(t, yb):
                # 32-partition block swap via SBUF->SBUF DMA (whole rows)
                for blk in range(4):
                    nc.sync.dma_start(
                        yb[blk * 32:(blk + 1) * 32, :],
                        t[(blk ^ 1) * 32:(blk ^ 1) * 32 + 32, :])

            def rope_chunk(t, yb, ch, mode="mixed"):
                c0, c1 = ch * 512, (ch + 1) * 512
                if mode == "pool":
                    # keep DVE free for attention mask adds: the whole
                    # rotation runs on the (otherwise idle) Pool engine
                    nc.gpsimd.tensor_tensor(yb[:, c0:c1], yb[:, c0:c1],
                                            stab[:, c0:c1],
                                            mybir.AluOpType.mult)
                    nc.gpsimd.tensor_tensor(t[:, c0:c1], t[:, c0:c1],
                                            ctab[:, c0:c1],
                                            mybir.AluOpType.mult)
                    nc.gpsimd.tensor_tensor(t[:, c0:c1], t[:, c0:c1],
                                            yb[:, c0:c1],
                                            mybir.AluOpType.add)
                    return
                if mode == "mixed":
                    nc.gpsimd.tensor_tensor(yb[:, c0:c1], yb[:, c0:c1],
                                            stab[:, c0:c1],
                                            mybir.AluOpType.mult)
                else:  # dve
                    nc.vector.tensor_mul(yb[:, c0:c1], yb[:, c0:c1],
                                         stab[:, c0:c1])
                nc.vector.tensor_mul(t[:, c0:c1], t[:, c0:c1],
                                     ctab[:, c0:c1])
                nc.vector.tensor_add(t[:, c0:c1], t[:, c0:c1],
                                     yb[:, c0:c1])

            def rope(t, idx, mode="mixed"):
                rope_swaps(t, ry)
                for ch in range(4):
                    rope_chunk(t, ry, ch, mode=mode)

            # ---- front: Q pair0 (DMA-pipelined), latent c, K pair0 ----
            front_gemm_qc(wq0, 0, qpair[0], "q0")
            rope(qpair[0], 0)
            front_gemm_kt(wdkv, DC, 0, cpair[0], "c0")
            front_gemm_kt(wdkv, DC, 128, cpair[1], "c1")
            front_gemm_kt(wupk, GD, 0, kpair[0], "k0", nkt=NCT, rhs=cpair)
            rope(kpair[0], 2, mode="dve")
            # PE fill while rope(k0) completes on DMA+DVE: all V tiles and
            # the pair-1 Q front (in-order PE stream keeps the array busy).

            def vaug_tile(st, act=True):
                # V tile: (s,d) natural layout + ones column, fp8.  The
                # PSUM -> SBUF copy goes to ACT pre-attention (idle then)
                # and to DVE when emitted inside the attention interleave
                # (GPSIMD cannot read PSUM).
                acc = psA.tile([128, 512], F32, tag="fa", name=f"vacc{st}")
                for kt in range(NCT):
                    nc.tensor.matmul(
                        acc[:, 0:GD],
                        cpair[kt][:, st * 128:(st + 1) * 128],
                        wupv[:, kt * GD:(kt + 1) * GD],
                        start=(kt == 0), stop=(kt == NCT - 1),
                    )
                eng = nc.scalar.copy if act else nc.vector.tensor_copy
                eng(vaug[:, st * 260:(st + 1) * 260].rearrange(
                        "p (h c) -> p h c", c=65)[:, :, 0:64],
                    acc[:, 0:GD].rearrange("p (h c) -> p h c", c=64))

            for st in range(8):
                vaug_tile(st)

            def attn_pair(pair, attn_t, mid=None, subfill=None, post_qc=None):
                qh = qpair[pair]
                kh = kpair[pair]
                for c in range(2):
                    if c == 1 and mid is not None:
                        mid()
                    c0, c1 = 1024 * c, 1024 * (c + 1)
                    for sub in range(2):
                        h = 2 * pair + sub
                        po = 64 * sub
                        # bank bb of ot finishes its causal accumulation at
                        # i == 4*bank+3; its reciprocal + opair copy are
                        # emitted right there so they overlap the other
                        # bank's PV matmuls instead of serializing at the
                        # (c, sub) boundary.
                        ot = psB.tile([128, 1024], F32, tag="ot",
                                      name=f"ot{h}_{c}")
                        vsl = vaug[:].rearrange("p (s c) -> p s c", c=260)
                        for j in range(NST // 2):
                            i0, i1 = 2 * j, 2 * j + 1
                            if c1 <= 128 * i0:
                                continue
                            # pt holds exp(scores) for the k-tile pair
                            # (i0, i1) side by side; the PV matmul runs in
                            # fp8 DoubleRow mode over both tiles at once.
                            pt = sbp.tile([128, 2048], FP8, tag="pt",
                                          name=f"pt{h}_{j}_{c}")
                            for ip, i in enumerate((i0, i1)):
                                qlo = 128 * i
                                if c1 <= qlo:
                                    continue
                                lo = max(qlo, c0)
                                st_t = psS.tile([128, 1024], F32, tag="st",
                                                name=f"stt{h}_{i}_{c}")
                                for bb in range(2):
                                    b0 = c0 + 512 * bb
                                    b1 = b0 + 512
                                    blo = max(lo, b0)
                                    if blo >= b1:
                                        continue
                                    diag = b0 <= qlo < b1
                                    nc.tensor.matmul(
                                        st_t[:, blo - c0:b1 - c0],
                                        kh[po:po + 64, i * 128:(i + 1) * 128],
                                        qh[po:po + 64, blo:b1],
                                        start=True, stop=not diag,
                                        skip_group_check=True,
                                    )
                                    if diag:
                                        # additive -inf causal mask folded
                                        # into the score accumulation group
                                        # on PE: ident^T @ maskneg adds
                                        # mask[k, q] to the diagonal block
                                        nc.tensor.matmul(
                                            st_t[:, qlo - c0:
                                                 qlo - c0 + 128],
                                            ident_t[:], mask_t[:],
                                            start=False, stop=True,
                                            skip_group_check=True,
                                        )
                                nc.scalar.activation(
                                    pt[:, ip * 1024 + lo - c0:
                                       (ip + 1) * 1024],
                                    st_t[:, lo - c0:1024],
                                    mybir.ActivationFunctionType.Exp,
                                    scale=float(1.0 / np.sqrt(DH)),
                                )
                            # strip: columns where only tile i0 contributes
                            slo, shi = max(128 * i0, c0), min(128 * i1, c1)
                            if slo < shi:
                                nc.tensor.matmul(
                                    ot[0:65, slo - c0:shi - c0],
                                    vsl[:, i0, 65 * h:65 * h + 65],
                                    pt[:, slo - c0:shi - c0],
                                    start=(j == 0), stop=False,
                                    skip_group_check=True,
                                )
                            plo = max(128 * i1, c0)
                            ptp = pt.rearrange("p (t n) -> p t n", t=2)
                            for bb in range(2):
                                b0 = c0 + 512 * bb
                                b1 = b0 + 512
                                blo = max(plo, b0)
                                if blo >= b1:
                                    continue
                                nc.tensor.matmul(
                                    ot[0:65, blo - c0:b1 - c0],
                                    vsl[:, i0:i0 + 2, 65 * h:65 * h + 65],
                                    ptp[:, :, blo - c0:b1 - c0],
                                    start=(j == 0),
                                    stop=(j == 4 * c + 2 * bb + 1),
                                    perf_mode=mybir.MatmulPerfMode.DoubleRow,
                                    skip_group_check=True,
                                )
                            if j == 4 * c + 1 or j == 4 * c + 3:
                                bb = (j // 2) % 2
                                with nc.allow_low_precision(
                                        reason="recip to f32r"):
                                    nc.vector.reciprocal(
                                        recip[32 * sub:32 * sub + 1,
                                              bb * 512:(bb + 1) * 512],
                                        ot[64:65, bb * 512:(bb + 1) * 512])
                                nc.vector.tensor_copy(
                                    opair[po:po + 64,
                                          bb * 512:(bb + 1) * 512],
                                    ot[0:64, bb * 512:(bb + 1) * 512])
                        if subfill is not None:
                            subfill(c, sub)
                    for qc in range(2):
                        rt = psA.tile([128, 512], F32, tag="fa",
                                      name=f"rtt{pair}_{c}_{qc}")
                        nc.tensor.matmul(
                            rt[:], ee_t[:],
                            recip[:, qc * 512:(qc + 1) * 512],
                            start=True, stop=True)
                        nc.vector.tensor_mul(
                            attn_t[:, c0 + qc * 512:c0 + (qc + 1) * 512],
                            opair[:, qc * 512:(qc + 1) * 512], rt[:])
                        if post_qc is not None:
                            post_qc(c, qc)

            def wo_gemm(qg_lo, qg_hi, act_ok=False):
                # combined: out = attn0^T @ W_O[0:128] + attn1^T @ W_O[128:]
                for qg in range(qg_lo, qg_hi):
                    ostage = sbo.tile([128, 2 * D], BF16, tag="ostage",
                                      name=f"ostage{qg}")
                    for qi in range(2):
                        qt = qg * 2 + qi
                        accs = [psA.tile([128, 512], F32, tag="fa",
                                         name=f"woacc{qt}_{n}")
                                for n in range(2)]
                        for p in range(2):
                            at = attn0 if p == 0 else attn1
                            for nch in range(2):
                                nc.tensor.matmul(
                                    accs[nch][:],
                                    at[:, qt * 128:(qt + 1) * 128],
                                    wo[:, p * D + nch * 512:
                                       p * D + (nch + 1) * 512],
                                    start=(p == 0), stop=(p == 1),
                                    skip_group_check=True,
                                )
                        for nch in range(2):
                            if nch == 1 and act_ok:
                                nc.scalar.copy(
                                    ostage[:, qi * D + 512:qi * D + 1024],
                                    accs[1][:])
                            else:
                                nc.vector.tensor_copy(
                                    ostage[:, qi * D + nch * 512:
                                           qi * D + (nch + 1) * 512],
                                    accs[nch][:])
                    nc.sync.dma_start(
                        out_d[qg * 256:(qg + 1) * 256, :].rearrange(
                            "(t p) n -> p t n", p=128),
                        ostage[:].rearrange("p (t n) -> p t n", t=2))

            def mid0():
                front_gemm_kt2(wupk, GD, 128, kpair[1], "k1", nkt=NCT,
                               rhs=cpair)
                rope_swaps(kpair[1], ry2)
                for ch in range(4):
                    rope_chunk(qpair[1], ry, ch, mode="pool")

            def fill0(c, sub):
                # PE bubble fillers inside pair-0 attention: the rest of the
                # V tiles, the pair-1 Q front, and the pair-1 ropes (Pool
                # engine) so they never delay DVE mask adds or ACT exps.
                if c == 0 and sub == 0:
                    for st in range(8, NST):
                        vaug_tile(st, act=False)
                elif c == 0 and sub == 1:
                    front_gemm_kt2(wq1, 128, 0, qpair[1], "q1")
                    rope_swaps(qpair[1], ry)
                elif c == 1 and sub == 0:
                    for ch in range(4):
                        rope_chunk(kpair[1], ry2, ch, mode="pool")

            attn_pair(0, attn0, mid=mid0, subfill=fill0)

            def post_qc1(c, qc):
                if c == 1 and qc == 0:
                    wo_gemm(4, 6)

            attn_pair(1, attn1, mid=lambda: wo_gemm(0, 4), post_qc=post_qc1)
            wo_gemm(6, 8, act_ok=True)

            sbo_cm.__exit__(None, None, None)
            sbp_cm.__exit__(None, None, None)
            sbf_cm.__exit__(None, None, None)

    nc.compile()
    return nc


_NC_CACHE = []


def _get_nc():
    if not _NC_CACHE:
        _NC_CACHE.append(_build_nc())
    return _NC_CACHE[0]


def _host_tables():
    theta = 10000.0 ** (-np.arange(0, DH, 2, dtype=np.float64) / DH)  # (32,)
    pos = np.arange(S, dtype=np.float64)
    ang = np.outer(theta, pos)  # (32, S)
    cos = np.cos(ang).astype(np.float32)
    sin = np.sin(ang).astype(np.float32)
    ctab = np.tile(cos, (4, 1))  # (128, S)
    stab = np.concatenate([-sin, sin, -sin, sin], axis=0).astype(np.float32)
    # additive causal mask on scores: 0 where q >= k, else -1e30 (the exp
    # then yields exactly 0, and pt can be stored in fp8)
    mask = np.where(np.arange(128)[None, :] >= np.arange(128)[:, None],
                    0.0, -1e30).astype(ml_dtypes.bfloat16)  # (128,128)
    # rt broadcast matrix: row 0 -> sub0 recip (partitions 0:64 of rt),
    # row 32 -> sub1 recip (partitions 64:128); rows must be 32-aligned.
    ee = np.zeros((64, 128), np.float32)
    ee[0, 0:64] = 1.0
    ee[32, 64:128] = 1.0
    ident = np.eye(128, dtype=np.float32).astype(ml_dtypes.bfloat16)
    return ctab, stab, mask, ident, ee


def _make_in_maps(X, W_Q, W_down_kv, W_up_K, W_up_V, W_O):
    X = np.asarray(X, np.float32)
    W_Q = np.asarray(W_Q, np.float32)
    W_down_kv = np.asarray(W_down_kv, np.float32)
    W_up_K = np.asarray(W_up_K, np.float32)
    W_up_V = np.asarray(W_up_V, np.float32)
    W_O = np.asarray(W_O, np.float32)

    ctab, stab, mask, ident, ee = _host_tables()
    bf = ml_dtypes.bfloat16
    ctab, stab = ctab.astype(bf), stab.astype(bf)
    # per-head even/odd de-interleave permutation (applied to W_Q and W_up_K
    # output columns; Q.K dot products are invariant under the shared perm)
    perm_head = np.concatenate([np.arange(0, DH, 2), np.arange(1, DH, 2)])

    xts = [np.ascontiguousarray(X[b].T).astype(bf) for b in range(B)]
    in_maps = []
    for c in range(N_CORES):
        b, g = c // 4, c % 4
        cols = np.concatenate(
            [(g * HPC + h) * DH + perm_head for h in range(HPC)])
        vcols = slice(g * GD, (g + 1) * GD)
        in_maps.append({
            "xt": xts[b],
            "wq": np.ascontiguousarray(W_Q[:, cols]).astype(bf),
            "wdkv": W_down_kv.astype(bf),
            "wupk": np.ascontiguousarray(W_up_K[:, cols]).astype(bf),
            "wupv": np.ascontiguousarray(W_up_V[:, vcols]).astype(bf),
            "wo": np.ascontiguousarray(W_O[vcols, :]).astype(bf),
            "ctab": ctab, "stab": stab, "mask": mask, "ident": ident,
            "ee": ee,
        })

    return in_maps


def _gather(res):
    out = np.zeros((B, S, D), np.float32)
    for c in range(N_CORES):
        out[c // 4] += res.results[c]["out"].astype(np.float32)
    return out


def kernel(X, W_Q, W_down_kv, W_up_K, W_up_V, W_O):
    in_maps = _make_in_maps(X, W_Q, W_down_kv, W_up_K, W_up_V, W_O)
    nc = _get_nc()
    res = run_bass_kernel_spmd(nc, in_maps, core_ids=list(range(N_CORES)))
    return _gather(res)
